# revision 29
# baseline (speedup 1.0000x reference)
"""DecoderLSTM Trainium2 kernel.

Computes, for inputs matching the reference nn module:
    x  = embed_table[captions]                      # [B, T, E]
    xg = einsum('bte,ge->tbg', x, W_ih) + b_ih + b_hh
    (h, c) LSTM scan over T steps, h0 = features, c0 = 0
    out = einsum('tbh,vh->btv', hs, W_out) + b_out  # [B, T, V]

Sharding: data-parallel over batch. 8 cores x 16 batch rows each.
Weights are replicated (cast to bf16 host-side); each core computes its
16-row slice of the output. Per-core output is produced in transposed
layout [V, T*Bc] and untransposed on the host during unshard.

Device layout notes (per core, Bc = 16 batch rows):
  - Embedding gather: dma_gather(transpose=True) pulls the 320 caption
    rows of the bf16 [V, 384]-padded table directly into x_T layout
    [128p=E-offset, 3=E-block, 384=(t,b) col]. Table column 383 is 1.0
    so row 383 of W_ihT carries (b_ih + b_hh): bias folded into the
    xg matmul.
  - Gate permutation: the 4H=2048 gate dim is reordered host-side so
    column-group j of the recurrent matmul computes
    [i_j | f_j | g_j | o_j] (H-slice j of each gate). Gates land in one
    PSUM bank as [128=(32j+b), 4, 128] and the whole nonlinearity runs
    on [128, *] tiles.
  - Recurrent matmul: 4 concurrent column-tiled matmuls (tile_position
    (0, 32j), M=16) stream W_hhT chunks; each group's accumulation is
    seeded by an identity-matmul that injects xg_t (start=True).
  - h_T for the next step comes from 4 row-tiled PE transposes of the
    [16@32k, 128] slices of h.
  - Projection: out_T = W_out @ hs_T with W_out blocks stationary,
    V on partitions; b_out added during PSUM evacuation via the ACT
    per-partition bias. Chunked in two so the first half overlaps the
    recurrence.
"""

import numpy as np
import ml_dtypes

import concourse.bass as bass
import concourse.mybir as mybir
import concourse.tile as tile
from concourse import bacc

BF16 = mybir.dt.bfloat16
F32 = mybir.dt.float32
I32 = mybir.dt.int32

B, T, E, H, V = 128, 20, 300, 512, 10000
EPAD = 384            # E padded; col 383 is the ones column (bias row)
NCORES = 8
BC = B // NCORES      # 16 batch rows per core
NT = BC * T           # 320 (t,b) columns per core
NIDX = 384            # gather idx count (padded to %128)
NV = 79               # ceil(10112 / 128) vocab row-tiles
VPAD = NV * 128       # 10112
AF = mybir.ActivationFunctionType


def _gate_perm():
    """new gate-dim order: chunk j = [i_j | f_j | g_j | o_j], blocks of 128."""
    perm = np.empty(4 * H, dtype=np.int64)
    n = 0
    for j in range(4):
        for q in range(4):          # i, f, g, o (PyTorch LSTM order)
            for r in range(128):
                perm[n] = q * H + j * 128 + r
                n += 1
    return perm


def build_nc():
    nc = bacc.Bacc("TRN2", target_bir_lowering=False, debug=False)

    # ---- DRAM parameters (per-core shapes) ----
    emb_d = nc.dram_tensor("emb", [V, EPAD], BF16, kind="ExternalInput")
    idx_d = nc.dram_tensor("idx32", [128, 3], I32, kind="ExternalInput")
    wih_d = nc.dram_tensor("wih", [3, 128, 4, 512], BF16, kind="ExternalInput")
    whh_d = nc.dram_tensor("whh", [4, 128, 4, 512], BF16, kind="ExternalInput")
    wout_d = nc.dram_tensor("wout", [4, 128, NV, 128], BF16, kind="ExternalInput")
    bout_d = nc.dram_tensor("bout", [128, NV], F32, kind="ExternalInput")
    h0t_d = nc.dram_tensor("h0t", [128, 4, BC], BF16, kind="ExternalInput")
    idf_d = nc.dram_tensor("idf", [128, 128], BF16, kind="ExternalInput")
    i16b_d = nc.dram_tensor("i16b", [16, 16], BF16, kind="ExternalInput")
    outT_d = nc.dram_tensor("outT", [128, NV, NT], BF16, kind="ExternalOutput")

    with tile.TileContext(nc) as tc:
        with (
            tc.tile_pool(name="const", bufs=1) as const,
            tc.tile_pool(name="wpool", bufs=1) as wpool,
            tc.tile_pool(name="xgstep", bufs=3) as xgstep_p,
            tc.tile_pool(name="work", bufs=2) as work,
            tc.tile_pool(name="stage", bufs=4) as stage_p,
            tc.tile_pool(name="psg", bufs=1, space="PSUM") as ps_gates,
            tc.tile_pool(name="psh", bufs=1, space="PSUM") as ps_ht,
            tc.tile_pool(name="psb", bufs=4, space="PSUM") as ps_big,
        ):
            idx_sb = const.tile([128, 3], I32, tag="idx")
            i16b_sb = const.tile([16, 16], BF16, tag="i16b")
            idf_sb = const.tile([128, 128], BF16, tag="idf")
            bout_sb = const.tile([128, NV], F32, tag="bout")

            # xT: [128p, m=(t,b)-chunk, k=E-block, 128=(t,b) col], e=128k+p
            xT = wpool.tile([128, 3, 3, 128], BF16, tag="xT")
            # xstage: gather landing pad, row r=128m+p of x at [p, m, :]
            xstage = wpool.tile([128, 3, EPAD], BF16, tag="xstage")
            wih_sb = [wpool.tile([128, 4, 512], BF16, tag=f"wih{k}", name=f"wih{k}") for k in range(3)]
            whh_sb = [wpool.tile([128, 4, 512], BF16, tag=f"whh{k}", name=f"whh{k}") for k in range(4)]
            wout_sb = [wpool.tile([128, NV, 128], BF16, tag=f"wo{k}", name=f"wo{k}") for k in range(4)]
            xg_sb = [wpool.tile([128, 4, 512], BF16, tag=f"xg{m}", name=f"xg{m}") for m in range(3)]
            # hs_T: slot s holds h after step s-1 (slot 0 = h0), packed
            # [128p=H-offset, slot, H-block k, b]
            hsT = wpool.tile([128, T + 1, 4, BC], BF16, tag="hsT")
            C = [wpool.tile([128, 128], F32, tag=f"C{c}", name=f"C{c}")
                 for c in range(2)]

            gates_ps = [ps_gates.tile([128, 4, 128], F32, tag=f"g{c}",
                                      name=f"g{c}") for c in range(2)]
            ht_ps = [ps_ht.tile([128, 4, 32], BF16, tag=f"ht{c}",
                                name=f"ht{c}") for c in range(2)]

            # ---- loads ----
            # sync HWDGE ring: only small latency-critical transfers (idx,
            # h0t, gather transposes, per-step xg rebases). scalar HWDGE
            # ring: bulk weights (wih/whh first, wout behind them).
            # Embedding gather: per-partition indirect DMA on the gpsimd
            # DGE (row idx[p,m] -> xstage[p,m,:]), then an XBAR dma
            # transpose into xT. This avoids dma_gather's custom-ucode
            # LOAD_LIB + op launch (~18us of startup in the old scheme).
            nc.sync.dma_start(idx_sb[:], idx_d[:])
            nc.sync.dma_start(hsT[:, 0, :, :], h0t_d[:])
            for m in range(3):
                nc.gpsimd.indirect_dma_start(
                    out=xstage[:, m, :], out_offset=None,
                    in_=emb_d[:],
                    in_offset=bass.IndirectOffsetOnAxis(
                        ap=idx_sb[:, m:m + 1], axis=0))
            for m in range(3):
                nc.sync.dma_start(xT[:, m, :, :], xstage[:, m, :],
                                  transpose=True)
            # wih and whh must BOTH land by ~step 0 (~28us); one HWDGE
            # queue only sustains ~115GB/s, so split the 3.7MB across the
            # scalar ring (wih + whh k0) and the gpsimd ring behind the
            # gathers (whh k1-3). sync stays clear for the latency-critical
            # transposes + per-step xg rebases. NOTHING that can park on a
            # semaphore goes on the scalar queue: a parked DMA issue blocks
            # the recurrence ACTIVATEs behind it (measured 47us stall).
            nc.scalar.dma_start(i16b_sb[:], i16b_d[:])
            for k in range(3):
                nc.scalar.dma_start(wih_sb[k][:], wih_d[k])
            nc.scalar.dma_start(whh_sb[0][:], whh_d[0])
            # wait_until floors keep the scheduler from hoisting these in
            # front of the (idx-blocked) gathers in the SWDGE stream
            with tc.tile_wait_until(0.008):
                for k in range(1, 4):
                    nc.gpsimd.dma_start(whh_sb[k][:], whh_d[k])
            nc.scalar.dma_start(idf_sb[:], idf_d[:])
            nc.scalar.dma_start(bout_sb[:], bout_d[:])
            for c in range(2):
                nc.vector.memset(gates_ps[c][:], 0.0)
                nc.vector.memset(C[c][:], 0.0)
            # wout rides the gpsimd SWDGE behind whh k1-3 (floored so it
            # cannot be hoisted in front of the gathers/whh); ~10MB at
            # SWDGE rate completes well before the projection (~110us).
            with tc.tile_wait_until(0.014):
                for k in range(4):
                    nc.gpsimd.dma_start(wout_sb[k][:], wout_d[k])

            # ---- xg = x @ W_ihT -> [(t,b) rows, 2048 perm'd gate cols] ----
            # m-tile 0 runs up front (needed at step 0); m1/m2 n-groups are
            # emitted inside steps 0..7 as PE filler during the act windows.
            def emit_xg(m, n):
                ps = ps_big.tile([128, 512], F32, tag="big")
                for k in range(3):
                    nc.tensor.matmul(
                        ps[:],
                        xT[:, m, k, :],
                        wih_sb[k][:, n, :],
                        start=(k == 0), stop=(k == 2),
                    )
                # evacuate on DVE only: the ACT queue carries the weight-DMA
                # issue instructions at kernel start, which would delay xg
                nc.vector.tensor_copy(xg_sb[m][:, n, :], ps[:])

            for n in range(4):
                emit_xg(0, n)

            HB = BC // 2  # 8 batch rows per pipelined chain

            def emit_xgt_fetch(c, t):
                # rebase chain-c step-t xg rows to partition base 0
                m, r0 = t // 8, (t % 8) * BC + c * HB
                xg_t = xgstep_p.tile([HB, 4, 512], BF16, tag=f"xgt{c}",
                                     name=f"xgt{c}_{t}")
                nc.sync.dma_start(xg_t[:], xg_sb[m][r0:r0 + HB, :, :])
                return xg_t

            def emit_mm(c, t, xg_t):
                """Gate matmuls for chain c step t."""
                gp = gates_ps[c]
                b0 = c * HB
                for j in range(4):
                    nc.tensor.matmul(
                        gp[32 * j:32 * j + HB, :, :],
                        i16b_sb[0:HB, 0:HB],
                        xg_t[:, j, :],
                        start=True, stop=False,
                        tile_position=(0, 32 * j),
                        skip_group_check=True,
                    )
                for k in range(4):
                    for j in range(4):
                        nc.tensor.matmul(
                            gp[32 * j:32 * j + HB, :, :],
                            hsT[:, t, k, b0:b0 + HB],
                            whh_sb[k][:, j, :],
                            start=False, stop=(k == 3),
                            tile_position=(0, 32 * j),
                            skip_group_check=True,
                        )

            def emit_tail(c, t):
                """Nonlinearity + h-transpose for chain c step t."""
                gp, hp, Cc = gates_ps[c], ht_ps[c], C[c]
                b0 = c * HB
                # nonlinearity: A = [sig(i), sig(f), tanh(g), sig(o)]
                A = work.tile([128, 4, 128], F32, tag=f"A{c}", name=f"A{c}_{t}")
                nc.scalar.activation(A[:, 0:2, :], gp[:, 0:2, :], AF.Sigmoid)
                nc.scalar.activation(A[:, 2, :], gp[:, 2, :], AF.Tanh)
                nc.scalar.activation(A[:, 3, :], gp[:, 3, :], AF.Sigmoid)
                T2 = work.tile([128, 128], F32, tag=f"T2{c}", name=f"T2{c}_{t}")
                T1 = work.tile([128, 128], F32, tag=f"T1{c}", name=f"T1{c}_{t}")
                TC = work.tile([128, 128], F32, tag=f"TC{c}", name=f"TC{c}_{t}")
                Hn = work.tile([128, 128], BF16, tag=f"Hn{c}", name=f"Hn{c}_{t}")
                nc.vector.tensor_mul(T2[:], A[:, 0, :], A[:, 2, :])   # i*g
                nc.vector.tensor_mul(T1[:], A[:, 1, :], Cc[:])        # f*c
                nc.vector.tensor_add(Cc[:], T1[:], T2[:])
                nc.scalar.activation(TC[:], Cc[:], AF.Tanh)
                nc.vector.tensor_mul(Hn[:], A[:, 3, :], TC[:])        # o*tanh(c)

                # h -> h_T: full 128x128 PE transpose (bf16 single-pass);
                # h_T blocks are hp[:, k, 0:8] (cols 8:32 are garbage)
                nc.tensor.transpose(hp[:], Hn[:], idf_sb[:])
                nc.vector.tensor_copy(hsT[:, t + 1, :, b0:b0 + HB],
                                      hp[:, :, 0:HB])

            # ---- recurrence: two half-batch chains, skewed pipeline ----
            # Emission order MM_A(t), tail_B(t-1), MM_B(t), tail_A(t) keeps
            # the PE FIFO free of transposes that wait on the not-yet-ready
            # nonlinearity while the other chain's matmuls could run.
            fetched = {(c, t): emit_xgt_fetch(c, t)
                       for t in (0, 1) for c in range(2)}
            for t in range(T):
                for c in range(2):
                    if (c, t + 2) not in fetched and t + 2 < T:
                        fetched[(c, t + 2)] = emit_xgt_fetch(c, t + 2)
                    emit_mm(c, t, fetched.pop((c, t)))
                    if c == 0:
                        if t > 0:
                            emit_tail(1, t - 1)
                    else:
                        emit_tail(0, t)
                # PE filler during the act windows: xg m1/m2 groups
                if t < 4:
                    emit_xg(1, t)
                elif t < 8:
                    emit_xg(2, t - 4)
            emit_tail(1, T - 1)

            # ---- projection tail: out_T = W_out @ hs_T, all 20 slots ----
            # (N=320 streams keep LDWEIGHTS fully hidden; PE is warm here)
            st = None
            for v in range(NV):
                pp = ps_big.tile([128, NT], F32, tag="big")
                for k in range(4):
                    nc.tensor.matmul(
                        pp[:],
                        wout_sb[k][:, v, :],
                        hsT[:, 1:T + 1, k, :],
                        start=(k == 0), stop=(k == 3),
                    )
                g = v % 4
                if g == 0:
                    nv = min(4, NV - v)
                    st = stage_p.tile([128, nv, NT], BF16, tag="st",
                                      name=f"st{v}")
                if v % 2 == 0:
                    nc.scalar.activation(st[:, g, :], pp[:], AF.Identity,
                                         bias=bout_sb[:, v:v + 1])
                else:
                    nc.vector.tensor_scalar_add(st[:, g, :], pp[:],
                                                bout_sb[:, v:v + 1])
                if g == 3 or v == NV - 1:
                    v0 = (v // 4) * 4
                    nc.sync.dma_start(outT_d[:, v0:v + 1, :], st[:])

    nc.compile()
    return nc


def prep_inputs(features, captions, embed_table, W_ih, W_hh, b_ih, b_hh,
                W_out, b_out):
    """Host-side shard + layout prep. Returns per-core input maps."""
    bf = ml_dtypes.bfloat16
    features = np.asarray(features, dtype=np.float32)
    captions = np.asarray(captions).astype(np.int64)
    embed_table = np.asarray(embed_table, dtype=np.float32)
    W_ih = np.asarray(W_ih, dtype=np.float32)
    W_hh = np.asarray(W_hh, dtype=np.float32)
    b_ih = np.asarray(b_ih, dtype=np.float32)
    b_hh = np.asarray(b_hh, dtype=np.float32)
    W_out = np.asarray(W_out, dtype=np.float32)
    b_out = np.asarray(b_out, dtype=np.float32)

    perm = _gate_perm()

    emb = np.zeros((V, EPAD), dtype=bf)
    emb[:, :E] = embed_table.astype(bf)
    emb[:, EPAD - 1] = bf(1.0)

    wih = np.zeros((EPAD, 4 * H), dtype=np.float32)
    wih[:E, :] = W_ih.T[:, perm]
    wih[EPAD - 1, :] = (b_ih + b_hh)[perm]
    wih = wih.astype(bf).reshape(3, 128, 4, 512)

    whh = np.ascontiguousarray(W_hh.T[:, perm]).astype(bf).reshape(4, 128, 4, 512)

    wout = np.zeros((H, VPAD), dtype=np.float32)
    wout[:, :V] = W_out.T
    wout = wout.astype(bf).reshape(4, 128, NV, 128)

    boutp = np.zeros((VPAD,), dtype=np.float32)
    boutp[:V] = b_out
    bout_r = np.ascontiguousarray(boutp.reshape(NV, 128).T)

    idf = np.eye(128, dtype=bf)
    i16b = np.eye(16, dtype=bf)

    shared = dict(emb=emb, wih=wih, whh=whh, wout=wout, bout=bout_r,
                  idf=idf, i16b=i16b)

    in_maps = []
    for c in range(NCORES):
        cap_c = captions[c * BC:(c + 1) * BC]                 # [16, 20]
        # per-partition gather indices: xstage row r=128m+p holds
        # x[(t=r//16, b=r%16)]; rows >= 320 gather row 0 (unused)
        r = np.arange(3 * 128)
        flat = np.where(r < NT, cap_c[np.minimum(r % BC, BC - 1),
                                      np.minimum(r // BC, T - 1)], 0)
        idx32 = np.ascontiguousarray(
            flat.reshape(3, 128).T.astype(np.int32))          # [128, 3]
        feat_c = features[c * BC:(c + 1) * BC]                # [16, 512]
        h0t = np.ascontiguousarray(
            feat_c.reshape(BC, 4, 128).transpose(2, 1, 0)).astype(bf)
        in_maps.append(dict(shared, idx32=idx32, h0t=h0t))
    return in_maps


def unshard(core_outs):
    """core_outs: list of 8 arrays [128, NV, NT] bf16 -> full [B, T, V] f32."""
    parts = []
    for o in core_outs:
        o = np.asarray(o, dtype=np.float32)          # [128, NV, NT]
        o = o.transpose(1, 0, 2).reshape(VPAD, NT)[:V]             # [V, 320]
        parts.append(o.reshape(V, T, BC).transpose(2, 1, 0))       # [16, T, V]
    return np.ascontiguousarray(np.concatenate(parts, axis=0))


_NC_CACHE = {}


def kernel(**inputs) -> np.ndarray:
    from concourse.bass_utils import run_bass_kernel_spmd

    if "nc" not in _NC_CACHE:
        _NC_CACHE["nc"] = build_nc()
    nc = _NC_CACHE["nc"]

    in_maps = prep_inputs(**inputs)
    res = run_bass_kernel_spmd(nc, in_maps, core_ids=list(range(NCORES)))
    return unshard([res.results[c]["outT"] for c in range(NCORES)])



# revision 44
# speedup vs baseline: 1.0879x; 1.0879x over previous
"""DecoderLSTM Trainium2 kernel.

Computes, for inputs matching the reference nn module:
    x  = embed_table[captions]                      # [B, T, E]
    xg = einsum('bte,ge->tbg', x, W_ih) + b_ih + b_hh
    (h, c) LSTM scan over T steps, h0 = features, c0 = 0
    out = einsum('tbh,vh->btv', hs, W_out) + b_out  # [B, T, V]

Sharding: data-parallel over batch. 8 cores x 16 batch rows each.
Weights are replicated (cast to bf16 host-side); each core computes its
16-row slice of the output. Per-core output is produced in transposed
layout [V, T*Bc] and untransposed on the host during unshard.

Device layout notes (per core, Bc = 16 batch rows):
  - Embedding gather: dma_gather(transpose=True) pulls the 320 caption
    rows of the bf16 [V, 384]-padded table directly into x_T layout
    [128p=E-offset, 3=E-block, 384=(t,b) col]. Table column 383 is 1.0
    so row 383 of W_ihT carries (b_ih + b_hh): bias folded into the
    xg matmul.
  - Gate permutation: the 4H=2048 gate dim is reordered host-side so
    column-group j of the recurrent matmul computes
    [i_j | f_j | g_j | o_j] (H-slice j of each gate). Gates land in one
    PSUM bank as [128=(32j+b), 4, 128] and the whole nonlinearity runs
    on [128, *] tiles.
  - Recurrent matmul: 4 concurrent column-tiled matmuls (tile_position
    (0, 32j), M=16) stream W_hhT chunks; each group's accumulation is
    seeded by an identity-matmul that injects xg_t (start=True).
  - h_T for the next step comes from 4 row-tiled PE transposes of the
    [16@32k, 128] slices of h.
  - Projection: out_T = W_out @ hs_T with W_out blocks stationary,
    V on partitions; b_out added during PSUM evacuation via the ACT
    per-partition bias. Chunked in two so the first half overlaps the
    recurrence.
"""

import numpy as np
import ml_dtypes

import concourse.bass as bass
import concourse.mybir as mybir
import concourse.tile as tile
from concourse import bacc

BF16 = mybir.dt.bfloat16
F32 = mybir.dt.float32
I32 = mybir.dt.int32

B, T, E, H, V = 128, 20, 300, 512, 10000
EPAD = 384            # E padded; col 383 is the ones column (bias row)
NCORES = 8
BC = B // NCORES      # 16 batch rows per core
NT = BC * T           # 320 (t,b) columns per core
NIDX = 384            # gather idx count (padded to %128)
NV = 79               # ceil(10112 / 128) vocab row-tiles
VPAD = NV * 128       # 10112
AF = mybir.ActivationFunctionType


def _gate_perm():
    """new gate-dim order: chunk j = [i_j | f_j | g_j | o_j], blocks of 128."""
    perm = np.empty(4 * H, dtype=np.int64)
    n = 0
    for j in range(4):
        for q in range(4):          # i, f, g, o (PyTorch LSTM order)
            for r in range(128):
                perm[n] = q * H + j * 128 + r
                n += 1
    return perm


def build_nc():
    nc = bacc.Bacc("TRN2", target_bir_lowering=False, debug=False)

    # ---- DRAM parameters (per-core shapes) ----
    emb_d = nc.dram_tensor("emb", [V, EPAD], BF16, kind="ExternalInput")
    idx_d = nc.dram_tensor("idx32", [128, 3], I32, kind="ExternalInput")
    wih_d = nc.dram_tensor("wih", [3, 128, 4, 512], BF16, kind="ExternalInput")
    whh_d = nc.dram_tensor("whh", [4, 128, 4, 512], BF16, kind="ExternalInput")
    wout_d = nc.dram_tensor("wout", [4, 128, NV, 128], BF16, kind="ExternalInput")
    bout_d = nc.dram_tensor("bout", [128, NV], F32, kind="ExternalInput")
    h0t_d = nc.dram_tensor("h0t", [128, 4, BC], BF16, kind="ExternalInput")
    idf_d = nc.dram_tensor("idf", [128, 128], BF16, kind="ExternalInput")
    # shifted identities: i32sh[32g + r, s, b] = 1 iff r == 8*s + b (same
    # content in each 32-row group). Inject stationary: lets the moving xg
    # slice sit at any 32-aligned partition base (stationary must share the
    # base) while selecting 8 rows at sub-offset 8*s.
    i16b_d = nc.dram_tensor("i32sh", [128, 4, 8], BF16, kind="ExternalInput")
    outT_d = nc.dram_tensor("outT", [128, NV, NT], BF16, kind="ExternalOutput")

    with tile.TileContext(nc) as tc:
        with (
            tc.tile_pool(name="const", bufs=1) as const,
            tc.tile_pool(name="wpool", bufs=1) as wpool,
            tc.tile_pool(name="work", bufs=2) as work,
            tc.tile_pool(name="stage", bufs=4) as stage_p,
            tc.tile_pool(name="psg", bufs=1, space="PSUM") as ps_gates,
            tc.tile_pool(name="psh", bufs=1, space="PSUM") as ps_ht,
            tc.tile_pool(name="psb", bufs=4, space="PSUM") as ps_big,
        ):
            idx_sb = const.tile([128, 3], I32, tag="idx")
            i16b_sb = const.tile([128, 4, 8], BF16, tag="i32sh")
            idf_sb = const.tile([128, 128], BF16, tag="idf")
            bout_sb = const.tile([128, NV], F32, tag="bout")

            # xT: [128p, m=(t,b)-chunk, k=E-block, 128=(t,b) col], e=128k+p
            xT = wpool.tile([128, 3, 3, 128], BF16, tag="xT")
            # xstage: gather landing pad, row r=128m+p of x at [p, m, :]
            xstage = wpool.tile([128, 3, EPAD], BF16, tag="xstage")
            wih_sb = [wpool.tile([128, 4, 512], BF16, tag=f"wih{k}", name=f"wih{k}") for k in range(3)]
            whh_sb = [wpool.tile([128, 4, 512], BF16, tag=f"whh{k}", name=f"whh{k}") for k in range(4)]
            wout_sb = [wpool.tile([128, NV, 128], BF16, tag=f"wo{k}", name=f"wo{k}") for k in range(4)]
            xg_sb = [wpool.tile([128, 4, 512], BF16, tag=f"xg{m}", name=f"xg{m}") for m in range(3)]
            # hs_T: slot s holds h after step s-1 (slot 0 = h0), packed
            # [128p=H-offset, slot, H-block k, b]
            hsT = wpool.tile([128, T + 1, 4, BC], BF16, tag="hsT")
            C = [wpool.tile([128, 128], F32, tag=f"C{c}", name=f"C{c}")
                 for c in range(2)]

            gates_ps = [ps_gates.tile([128, 4, 128], F32, tag=f"g{c}",
                                      name=f"g{c}") for c in range(2)]
            ht_ps = [ps_ht.tile([128, 4, 32], BF16, tag=f"ht{c}",
                                name=f"ht{c}") for c in range(2)]

            # ---- loads ----
            # sync HWDGE ring: only small latency-critical transfers (idx,
            # h0t, gather transposes, per-step xg rebases). scalar HWDGE
            # ring: bulk weights (wih/whh first, wout behind them).
            # Embedding gather: per-partition indirect DMA on the gpsimd
            # DGE (row idx[p,m] -> xstage[p,m,:]), then an XBAR dma
            # transpose into xT. This avoids dma_gather's custom-ucode
            # LOAD_LIB + op launch (~18us of startup in the old scheme).
            nc.sync.dma_start(idx_sb[:], idx_d[:])
            nc.sync.dma_start(hsT[:, 0, :, :], h0t_d[:])
            for m in range(3):
                nc.gpsimd.indirect_dma_start(
                    out=xstage[:, m, :], out_offset=None,
                    in_=emb_d[:],
                    in_offset=bass.IndirectOffsetOnAxis(
                        ap=idx_sb[:, m:m + 1], axis=0))
            for m in range(3):
                nc.sync.dma_start(xT[:, m, :, :], xstage[:, m, :],
                                  transpose=True)
            # ALL bulk weights ride the gpsimd SWDGE (measured ~350GB/s)
            # behind the gathers. The scalar engine queue carries ZERO DMA
            # issues: any DMA issue can park on a semaphore-reuse wait and
            # block the recurrence ACTIVATEs + sigmoid table load behind it
            # (measured 47us stall). wait_until floors keep the scheduler
            # from hoisting the dep-free weight loads in front of the
            # (idx-blocked) gathers in the SWDGE stream.
            with tc.tile_wait_until(0.006):
                nc.gpsimd.dma_start(i16b_sb[:], i16b_d[:])
                nc.gpsimd.dma_start(idf_sb[:], idf_d[:])
            with tc.tile_wait_until(0.008):
                for k in range(3):
                    nc.gpsimd.dma_start(wih_sb[k][:], wih_d[k])
            with tc.tile_wait_until(0.010):
                for k in range(4):
                    nc.gpsimd.dma_start(whh_sb[k][:], whh_d[k])
            with tc.tile_wait_until(0.012):
                nc.gpsimd.dma_start(bout_sb[:], bout_d[:])
            for c in range(2):
                nc.vector.memset(gates_ps[c][:], 0.0)
                nc.vector.memset(C[c][:], 0.0)
            # wout last on the SWDGE: ~10MB completes ~60us, well before
            # the projection needs it (~110us).
            with tc.tile_wait_until(0.014):
                for k in range(4):
                    nc.gpsimd.dma_start(wout_sb[k][:], wout_d[k])

            # ---- xg = x @ W_ihT -> [(t,b) rows, 2048 perm'd gate cols] ----
            # m-tile 0 runs up front (needed at step 0); m1/m2 n-groups are
            # emitted inside steps 0..7 as PE filler during the act windows.
            def emit_xg(m, n):
                ps = ps_big.tile([128, 512], F32, tag="big")
                for k in range(3):
                    nc.tensor.matmul(
                        ps[:],
                        xT[:, m, k, :],
                        wih_sb[k][:, n, :],
                        start=(k == 0), stop=(k == 2),
                    )
                # evacuate on DVE only: the ACT queue carries the weight-DMA
                # issue instructions at kernel start, which would delay xg
                nc.vector.tensor_copy(xg_sb[m][:, n, :], ps[:])

            for n in range(4):
                emit_xg(0, n)

            HB = BC // 2  # 8 batch rows per pipelined chain

            def emit_mm(c, t):
                """Gate matmuls for chain c step t. The xg inject streams
                straight out of xg_sb (no rebase DMA): moving base must be
                32-aligned, so stream the aligned 32-row window and select
                rows r0..r0+8 via the shifted-identity stationary."""
                gp = gates_ps[c]
                b0 = c * HB
                m, r0 = t // 8, (t % 8) * BC + c * HB
                base, s = r0 & ~31, (r0 % 32) // 8
                for j in range(4):
                    nc.tensor.matmul(
                        gp[32 * j:32 * j + HB, :, :],
                        i16b_sb[base:base + 32, s, :],
                        xg_sb[m][base:base + 32, j, :],
                        start=True, stop=False,
                        tile_position=(base, 32 * j),
                        skip_group_check=True,
                    )
                for k in range(4):
                    for j in range(4):
                        nc.tensor.matmul(
                            gp[32 * j:32 * j + HB, :, :],
                            hsT[:, t, k, b0:b0 + HB],
                            whh_sb[k][:, j, :],
                            start=False, stop=(k == 3),
                            tile_position=(0, 32 * j),
                            skip_group_check=True,
                        )

            def emit_tail(c, t):
                """Nonlinearity + h-transpose for chain c step t."""
                gp, hp, Cc = gates_ps[c], ht_ps[c], C[c]
                b0 = c * HB
                # nonlinearity: A = [sig(i), sig(f), tanh(g), sig(o)]
                A = work.tile([128, 4, 128], F32, tag=f"A{c}", name=f"A{c}_{t}")
                nc.scalar.activation(A[:, 0:2, :], gp[:, 0:2, :], AF.Sigmoid)
                nc.scalar.activation(A[:, 2, :], gp[:, 2, :], AF.Tanh)
                nc.scalar.activation(A[:, 3, :], gp[:, 3, :], AF.Sigmoid)
                T2 = work.tile([128, 128], F32, tag=f"T2{c}", name=f"T2{c}_{t}")
                T1 = work.tile([128, 128], F32, tag=f"T1{c}", name=f"T1{c}_{t}")
                TC = work.tile([128, 128], F32, tag=f"TC{c}", name=f"TC{c}_{t}")
                Hn = work.tile([128, 128], BF16, tag=f"Hn{c}", name=f"Hn{c}_{t}")
                nc.vector.tensor_mul(T2[:], A[:, 0, :], A[:, 2, :])   # i*g
                nc.vector.tensor_mul(T1[:], A[:, 1, :], Cc[:])        # f*c
                nc.vector.tensor_add(Cc[:], T1[:], T2[:])
                nc.scalar.activation(TC[:], Cc[:], AF.Tanh)
                nc.vector.tensor_mul(Hn[:], A[:, 3, :], TC[:])        # o*tanh(c)

                # h -> h_T: full 128x128 PE transpose (bf16 single-pass);
                # h_T blocks are hp[:, k, 0:8] (cols 8:32 are garbage)
                nc.tensor.transpose(hp[:], Hn[:], idf_sb[:])
                nc.vector.tensor_copy(hsT[:, t + 1, :, b0:b0 + HB],
                                      hp[:, :, 0:HB])

            # ---- recurrence: two half-batch chains, skewed pipeline ----
            # Emission order MM_A(t), tail_B(t-1), MM_B(t), tail_A(t) keeps
            # the PE FIFO free of transposes that wait on the not-yet-ready
            # nonlinearity while the other chain's matmuls could run.
            for t in range(T):
                for c in range(2):
                    emit_mm(c, t)
                    if c == 0:
                        if t > 0:
                            emit_tail(1, t - 1)
                    else:
                        emit_tail(0, t)
                # PE filler during the act windows: xg m1/m2 groups
                if t < 4:
                    emit_xg(1, t)
                elif t < 8:
                    emit_xg(2, t - 4)
            emit_tail(1, T - 1)

            # ---- projection tail: out_T = W_out @ hs_T, all 20 slots ----
            # (N=320 streams keep LDWEIGHTS fully hidden; PE is warm here)
            st = None
            for v in range(NV):
                pp = ps_big.tile([128, NT], F32, tag="big")
                for k in range(4):
                    nc.tensor.matmul(
                        pp[:],
                        wout_sb[k][:, v, :],
                        hsT[:, 1:T + 1, k, :],
                        start=(k == 0), stop=(k == 3),
                    )
                g = v % 4
                if g == 0:
                    nv = min(4, NV - v)
                    st = stage_p.tile([128, nv, NT], BF16, tag="st",
                                      name=f"st{v}")
                if v % 2 == 0:
                    nc.scalar.activation(st[:, g, :], pp[:], AF.Identity,
                                         bias=bout_sb[:, v:v + 1])
                else:
                    nc.vector.tensor_scalar_add(st[:, g, :], pp[:],
                                                bout_sb[:, v:v + 1])
                if g == 3 or v == NV - 1:
                    v0 = (v // 4) * 4
                    nc.sync.dma_start(outT_d[:, v0:v + 1, :], st[:])

    nc.compile()
    return nc


def prep_inputs(features, captions, embed_table, W_ih, W_hh, b_ih, b_hh,
                W_out, b_out):
    """Host-side shard + layout prep. Returns per-core input maps."""
    bf = ml_dtypes.bfloat16
    features = np.asarray(features, dtype=np.float32)
    captions = np.asarray(captions).astype(np.int64)
    embed_table = np.asarray(embed_table, dtype=np.float32)
    W_ih = np.asarray(W_ih, dtype=np.float32)
    W_hh = np.asarray(W_hh, dtype=np.float32)
    b_ih = np.asarray(b_ih, dtype=np.float32)
    b_hh = np.asarray(b_hh, dtype=np.float32)
    W_out = np.asarray(W_out, dtype=np.float32)
    b_out = np.asarray(b_out, dtype=np.float32)

    perm = _gate_perm()

    emb = np.zeros((V, EPAD), dtype=bf)
    emb[:, :E] = embed_table.astype(bf)
    emb[:, EPAD - 1] = bf(1.0)

    wih = np.zeros((EPAD, 4 * H), dtype=np.float32)
    wih[:E, :] = W_ih.T[:, perm]
    wih[EPAD - 1, :] = (b_ih + b_hh)[perm]
    wih = wih.astype(bf).reshape(3, 128, 4, 512)

    whh = np.ascontiguousarray(W_hh.T[:, perm]).astype(bf).reshape(4, 128, 4, 512)

    wout = np.zeros((H, VPAD), dtype=np.float32)
    wout[:, :V] = W_out.T
    wout = wout.astype(bf).reshape(4, 128, NV, 128)

    boutp = np.zeros((VPAD,), dtype=np.float32)
    boutp[:V] = b_out
    bout_r = np.ascontiguousarray(boutp.reshape(NV, 128).T)

    idf = np.eye(128, dtype=bf)
    i32sh = np.zeros((128, 4, 8), dtype=bf)
    for g in range(4):
        for s in range(4):
            for b in range(8):
                i32sh[32 * g + 8 * s + b, s, b] = bf(1.0)

    shared = dict(emb=emb, wih=wih, whh=whh, wout=wout, bout=bout_r,
                  idf=idf, i32sh=i32sh)

    in_maps = []
    for c in range(NCORES):
        cap_c = captions[c * BC:(c + 1) * BC]                 # [16, 20]
        # per-partition gather indices: xstage row r=128m+p holds
        # x[(t=r//16, b=r%16)]; rows >= 320 gather row 0 (unused)
        r = np.arange(3 * 128)
        flat = np.where(r < NT, cap_c[np.minimum(r % BC, BC - 1),
                                      np.minimum(r // BC, T - 1)], 0)
        idx32 = np.ascontiguousarray(
            flat.reshape(3, 128).T.astype(np.int32))          # [128, 3]
        feat_c = features[c * BC:(c + 1) * BC]                # [16, 512]
        h0t = np.ascontiguousarray(
            feat_c.reshape(BC, 4, 128).transpose(2, 1, 0)).astype(bf)
        in_maps.append(dict(shared, idx32=idx32, h0t=h0t))
    return in_maps


def unshard(core_outs):
    """core_outs: list of 8 arrays [128, NV, NT] bf16 -> full [B, T, V] f32."""
    parts = []
    for o in core_outs:
        o = np.asarray(o, dtype=np.float32)          # [128, NV, NT]
        o = o.transpose(1, 0, 2).reshape(VPAD, NT)[:V]             # [V, 320]
        parts.append(o.reshape(V, T, BC).transpose(2, 1, 0))       # [16, T, V]
    return np.ascontiguousarray(np.concatenate(parts, axis=0))


_NC_CACHE = {}


def kernel(**inputs) -> np.ndarray:
    from concourse.bass_utils import run_bass_kernel_spmd

    if "nc" not in _NC_CACHE:
        _NC_CACHE["nc"] = build_nc()
    nc = _NC_CACHE["nc"]

    in_maps = prep_inputs(**inputs)
    res = run_bass_kernel_spmd(nc, in_maps, core_ids=list(range(NCORES)))
    return unshard([res.results[c]["outT"] for c in range(NCORES)])

